# revision 10
# baseline (speedup 1.0000x reference)
"""VQ codebook EMA update kernel for 8 Trainium2 NeuronCores.

Strategy (data-parallel over N):
  - each core gets N/8 = 2500 samples (padded to 2560 with zero-weight rows)
  - phase 1 (distances): argmin_k ||x-c_k||^2 == argmax_k (x.c_k - |c_k|^2/2).
    The matmul runs as a bf16 hi/lo split (xh*ch + xl*ch + xh*cl) which is
    fp32-grade accurate; -c2/2 is folded in as a 3-row bf16 rank-update so
    no fp32 matmuls are needed.  PSUM accumulates in fp32.  argmax via DVE
    max/max_index over [128, 4096] score rows.
  - phase 2 (segment sums): for each 128-cluster tile, accumulate
    sum_n onehot(idx)^T @ [w*x | w] over all 20 sample chunks in PSUM.
    The one-hot blocks are rebuilt on the fly from the stored indices
    (DVE is_equal against an iota row), and the matmuls run in float32r
    (full speed at free>=256, ~2^-14 accuracy).
  - cross-core: ReduceScatter(add) of [4096, 769] partial sums; each core
    then does the EMA update for its K/8 = 512 cluster slice.
"""

import numpy as np

N, K, D = 20000, 4096, 768
DECAY = 0.99
NCORES = 8
P = 128
NS = 2560                     # padded shard rows (20 * 128)
NCH = NS // P                 # 20
DC = D // P                   # 6
DW = D + 2                    # payload width: [w*x | w | w] (padded even for fp32r matmul)

_CACHE = {}


def _build(n_cores):
    import concourse.mybir as mybir
    import concourse.tile as tile
    from concourse import bacc

    dt = mybir.dt
    f32, bf16 = dt.float32, dt.bfloat16
    f32r = dt.float32r
    op = mybir.AluOpType

    ks = K // n_cores
    nc = bacc.Bacc(
        "TRN2", target_bir_lowering=False, debug=False, num_devices=n_cores
    )

    xt = nc.dram_tensor("xt", [D, NS], f32, kind="ExternalInput").ap()
    xn = nc.dram_tensor("xn", [NS, D], f32, kind="ExternalInput").ap()
    wv = nc.dram_tensor("wv", [NS], f32, kind="ExternalInput").ap()
    ct = nc.dram_tensor("ct", [D, K], f32, kind="ExternalInput").ap()
    cn = nc.dram_tensor("cn", [K, D], f32, kind="ExternalInput").ap()
    cm = nc.dram_tensor("cm", [ks, D], f32, kind="ExternalInput").ap()
    km = nc.dram_tensor("km", [ks], f32, kind="ExternalInput").ap()

    assign_o = nc.dram_tensor("assign_o", [NS], dt.int32, kind="ExternalOutput").ap()
    centers_o = nc.dram_tensor("centers_o", [ks, D], f32, kind="ExternalOutput").ap()
    counts_o = nc.dram_tensor("counts_o", [ks], f32, kind="ExternalOutput").ap()

    with tile.TileContext(nc) as tc:
        with (
            tc.tile_pool(name="const", bufs=1) as constp,
            tc.tile_pool(name="big", bufs=2) as bigp,
            tc.tile_pool(name="xs", bufs=2) as xsp,
            tc.tile_pool(name="xb", bufs=3) as xbp,
            tc.tile_pool(name="sm", bufs=3) as smp,
            tc.tile_pool(name="ps", bufs=2, space="PSUM") as psp,
            tc.tile_pool(name="dram", bufs=1, space="DRAM") as dramp,
        ):
            # ------------- DRAM scratch -------------
            seg_d = dramp.tile([K, DW], f32)
            xw_d = dramp.tile([NCH, P, DW], f32r)
            rs_d = dramp.tile([ks, DW], f32)

            # ------------- constants -------------
            ones3 = constp.tile([3, P], bf16)
            nc.gpsimd.memset(ones3[:], 1.0)
            # iota row 0..127 along free, same on every partition
            iota128 = constp.tile([P, P], dt.int32)
            nc.gpsimd.iota(
                iota128[:], pattern=[[1, P]], base=0, channel_multiplier=0
            )
            iotaf = constp.tile([P, P], f32)
            nc.vector.tensor_copy(iotaf[:], iota128[:])
            # per-chunk argmax indices, kept on-chip for phase 2
            idxcols = constp.tile([P, NCH], f32)

            with tc.tile_pool(name="cw", bufs=1) as cwp:
                # ---- C^T -> bf16 hi/lo split
                ch_t = cwp.tile([P, DC, K], bf16)
                cl_t = cwp.tile([P, DC, K], bf16)
                for dc in range(DC):
                    cts = bigp.tile([P, K], f32, tag="big")
                    nc.sync.dma_start(cts[:], ct[dc * P : (dc + 1) * P, :])
                    nc.scalar.copy(ch_t[:, dc, :], cts[:])
                    nc.vector.tensor_tensor(
                        cl_t[:, dc, :], cts[:], ch_t[:, dc, :], op.subtract
                    )

                # ---- c2 = row norms of centers (column layout)
                c2cols = constp.tile([P, 32], f32)
                for kc in range(32):
                    cst = xsp.tile([P, D], f32, tag="xnt")
                    nc.sync.dma_start(cst[:], cn[kc * P : (kc + 1) * P, :])
                    scr = xbp.tile([P, D], bf16, tag="xh")
                    nc.scalar.activation(
                        out=scr[:],
                        in_=cst[:],
                        func=mybir.ActivationFunctionType.Square,
                        accum_out=c2cols[:, kc : kc + 1],
                    )
                # split -0.5*c2 into 3 bf16 parts, assemble [3, K] row tile
                # via a small DRAM roundtrip
                c2hl_d = dramp.tile([3, K], bf16)
                c2t = constp.tile([P, 32], f32)
                c2d1 = constp.tile([P, 32], f32)
                c2d2 = constp.tile([P, 32], f32)
                chc = constp.tile([P, 32], bf16)
                cmc = constp.tile([P, 32], bf16)
                clc = constp.tile([P, 32], bf16)
                c2hl = constp.tile([3, K], bf16)
                nc.vector.tensor_scalar_mul(c2t[:], c2cols[:], -0.5)
                nc.vector.tensor_copy(chc[:], c2t[:])
                nc.vector.tensor_tensor(c2d1[:], c2t[:], chc[:], op.subtract)
                nc.vector.tensor_copy(cmc[:], c2d1[:])
                nc.vector.tensor_tensor(c2d2[:], c2d1[:], cmc[:], op.subtract)
                nc.vector.tensor_copy(clc[:], c2d2[:])
                for row, colt in ((0, chc), (1, cmc), (2, clc)):
                    nc.sync.dma_start(
                        c2hl_d[row, :].rearrange("(c p) -> p c", p=P), colt[:]
                    )
                nc.sync.dma_start(c2hl[:], c2hl_d[:])

                xtr = xt[:].rearrange("(c p) n -> p c n", p=P)

                # ---- phase 1: distances + argmax per 128-sample chunk
                for i in range(NCH):
                    nsl = slice(i * P, (i + 1) * P)

                    xts = xsp.tile([P, DC, P], f32, tag="xts")
                    nc.sync.dma_start(xts[:], xtr[:, :, nsl])
                    xh = xbp.tile([P, DC, P], bf16, tag="xh")
                    xl = xbp.tile([P, DC, P], bf16, tag="xl")
                    nc.scalar.copy(xh[:], xts[:])
                    nc.vector.tensor_tensor(xl[:], xts[:], xh[:], op.subtract)

                    wcol = smp.tile([P, 1], f32, tag="wcol")
                    nc.sync.dma_start(
                        wcol[:], wv[nsl].rearrange("(p a) -> p a", a=1)
                    )
                    xnt = xsp.tile([P, D], f32, tag="xnt")
                    nc.sync.dma_start(xnt[:], xn[nsl, :])
                    xw = xbp.tile([P, DW], f32r, tag="xw")
                    nc.vector.tensor_scalar_mul(xw[:, 0:D], xnt[:], wcol[:, 0:1])
                    nc.vector.tensor_copy(xw[:, D : D + 1], wcol[:])
                    nc.vector.tensor_copy(xw[:, D + 1 : D + 2], wcol[:])
                    nc.sync.dma_start(xw_d[i], xw[:])

                    s_sb = bigp.tile([P, K], f32, tag="big")
                    for q in range(4):
                        pst = psp.tile([P, 1024], f32, tag="ps")
                        for pd in range(19):
                            if pd < 18:
                                pi, dc = divmod(pd, DC)
                                lhsT = (xh if pi in (0, 2) else xl)[:, dc, :]
                                rt = ch_t if pi in (0, 1) else cl_t
                                rhs_of = lambda j: rt[
                                    :, dc, (q * 2 + j) * 512 : (q * 2 + j + 1) * 512
                                ]
                            else:
                                lhsT = ones3[:]
                                rhs_of = lambda j: c2hl[
                                    :, (q * 2 + j) * 512 : (q * 2 + j + 1) * 512
                                ]
                            for j in range(2):
                                nc.tensor.matmul(
                                    pst[:, j * 512 : (j + 1) * 512],
                                    lhsT,
                                    rhs_of(j),
                                    start=(pd == 0),
                                    stop=(pd == 18),
                                )
                        for j in range(2):
                            nc.scalar.copy(
                                s_sb[
                                    :, (q * 2 + j) * 512 : (q * 2 + j + 1) * 512
                                ],
                                pst[:, j * 512 : (j + 1) * 512],
                            )

                    vmax = smp.tile([P, 8], f32, tag="vmax")
                    vidx = smp.tile([P, 8], dt.uint16, tag="vidx")
                    nc.vector.max(vmax[:], s_sb[:])
                    nc.vector.max_index(vidx[:], vmax[:], s_sb[:])

                    a32 = smp.tile([P, 1], dt.int32, tag="a32")
                    nc.vector.tensor_copy(a32[:], vidx[:, 0:1])
                    nc.sync.dma_start(
                        assign_o[nsl].rearrange("(p a) -> p a", a=1), a32[:]
                    )
                    nc.vector.tensor_copy(
                        idxcols[:, i : i + 1], vidx[:, 0:1]
                    )

            # ---- phase 2: segment sums via one-hot f32r matmuls
            with (
                tc.tile_pool(name="p2one", bufs=1) as p2o,
                tc.tile_pool(name="p2sm", bufs=3) as p2s,
                tc.tile_pool(name="ema", bufs=1) as emap,
            ):
                xwres = p2o.tile([P, NCH, DW], f32r, tag="xwres")
                nc.sync.dma_start(
                    xwres[:], xw_d[:].rearrange("n p f -> p n f")
                )
                for kt in range(32):
                    cps = psp.tile([P, 1024], f32, tag="ps")
                    for n in range(NCH):
                        idq = p2s.tile([P, 1], f32, tag="idq")
                        nc.vector.tensor_scalar_add(
                            idq[:], idxcols[:, n : n + 1], float(-kt * P)
                        )
                        ablk = p2s.tile([P, P], f32r, tag="ablk")
                        nc.vector.tensor_scalar(
                            ablk[:], iotaf[:], idq[:, 0:1], None, op.is_equal
                        )
                        nc.tensor.matmul(
                            cps[:, 0:512],
                            ablk[:],
                            xwres[:, n, 0:512],
                            start=(n == 0),
                            stop=(n == NCH - 1),
                        )
                        nc.tensor.matmul(
                            cps[:, 512:DW],
                            ablk[:],
                            xwres[:, n, 512:DW],
                            start=(n == 0),
                            stop=(n == NCH - 1),
                        )
                    soup = p2s.tile([P, DW], f32, tag="soup")
                    nc.scalar.copy(soup[:], cps[:, 0:DW])
                    nc.sync.dma_start(seg_d[kt * P : (kt + 1) * P, :], soup[:])

                # ---- cross-core reduce
                nc.gpsimd.collective_compute(
                    "ReduceScatter",
                    op.add,
                    replica_groups=[list(range(n_cores))],
                    ins=[seg_d[:]],
                    outs=[rs_d[:]],
                )

                # ---- EMA update on this core's K/8 slice
                for t in range(ks // P):
                    ksl = slice(t * P, (t + 1) * P)
                    sws = emap.tile([P, D], f32, tag="sws")
                    nc.sync.dma_start(sws[:], rs_d[ksl, 0:D])
                    cen = emap.tile([P, D], f32, tag="cen")
                    nc.sync.dma_start(cen[:], cm[ksl, :])
                    sw = emap.tile([P, 1], f32, tag="sw")
                    nc.sync.dma_start(sw[:], rs_d[ksl, D : D + 1])
                    cnt = emap.tile([P, 1], f32, tag="cnt")
                    nc.sync.dma_start(
                        cnt[:], km[ksl].rearrange("(p a) -> p a", a=1)
                    )

                    swc = emap.tile([P, 1], f32, tag="swc")
                    nc.vector.tensor_scalar_max(swc[:], sw[:], 1e-12)
                    rec = emap.tile([P, 1], f32, tag="rec")
                    nc.vector.reciprocal(rec[:], swc[:])
                    newc = emap.tile([P, D], f32, tag="newc")
                    nc.vector.tensor_scalar_mul(newc[:], sws[:], rec[:, 0:1])

                    t1 = emap.tile([P, 1], f32, tag="t1")
                    nc.vector.tensor_scalar_add(t1[:], cnt[:], 1.0)
                    al = emap.tile([P, 1], f32, tag="al")
                    nc.vector.reciprocal(al[:], t1[:])
                    mc = emap.tile([P, 1], f32, tag="mc")
                    nc.vector.tensor_scalar(mc[:], cnt[:], 0.0, None, op.is_gt)
                    dal = emap.tile([P, 1], f32, tag="dal")
                    nc.vector.tensor_scalar_add(dal[:], al[:], -1.0)
                    m2 = emap.tile([P, 1], f32, tag="m2")
                    nc.vector.tensor_tensor(m2[:], dal[:], mc[:], op.mult)
                    alpha = emap.tile([P, 1], f32, tag="alpha")
                    nc.vector.tensor_scalar_add(alpha[:], m2[:], 1.0)
                    onem = emap.tile([P, 1], f32, tag="onem")
                    nc.vector.tensor_scalar(
                        onem[:], alpha[:], -1.0, 1.0, op.mult, op.add
                    )

                    p1 = emap.tile([P, D], f32, tag="p1")
                    nc.vector.tensor_scalar_mul(p1[:], newc[:], alpha[:, 0:1])
                    upd = emap.tile([P, D], f32, tag="upd")
                    nc.vector.scalar_tensor_tensor(
                        upd[:], cen[:], onem[:, 0:1], p1[:], op.mult, op.add
                    )

                    has = emap.tile([P, 1], f32, tag="has")
                    nc.vector.tensor_scalar(has[:], sw[:], 0.0, None, op.is_gt)
                    dd = emap.tile([P, D], f32, tag="dd")
                    nc.vector.tensor_tensor(dd[:], upd[:], cen[:], op.subtract)
                    md = emap.tile([P, D], f32, tag="md")
                    nc.vector.tensor_scalar_mul(md[:], dd[:], has[:, 0:1])
                    cout = emap.tile([P, D], f32, tag="cout")
                    nc.vector.tensor_tensor(cout[:], md[:], cen[:], op.add)
                    nc.sync.dma_start(centers_o[ksl, :], cout[:])

                    c99 = emap.tile([P, 1], f32, tag="c99")
                    nc.vector.tensor_scalar_mul(c99[:], cnt[:], DECAY)
                    t3 = emap.tile([P, 1], f32, tag="t3")
                    nc.vector.tensor_tensor(t3[:], c99[:], sw[:], op.add)
                    dcn = emap.tile([P, 1], f32, tag="dcn")
                    nc.vector.tensor_tensor(dcn[:], t3[:], cnt[:], op.subtract)
                    mdc = emap.tile([P, 1], f32, tag="mdc")
                    nc.vector.tensor_tensor(mdc[:], dcn[:], has[:], op.mult)
                    kout = emap.tile([P, 1], f32, tag="kout")
                    nc.vector.tensor_tensor(kout[:], mdc[:], cnt[:], op.add)
                    nc.sync.dma_start(
                        counts_o[ksl].rearrange("(p a) -> p a", a=1), kout[:]
                    )

    return nc


def _get_nc(n_cores=NCORES):
    if n_cores not in _CACHE:
        nc = _build(n_cores)
        nc.compile()
        _CACHE[n_cores] = nc
    return _CACHE[n_cores]


def _in_maps(X, centers, counts, sample_weight, n_cores=NCORES):
    X = np.ascontiguousarray(np.asarray(X, dtype=np.float32))
    centers = np.ascontiguousarray(np.asarray(centers, dtype=np.float32))
    counts = np.ascontiguousarray(np.asarray(counts, dtype=np.float32))
    sample_weight = np.ascontiguousarray(np.asarray(sample_weight, dtype=np.float32))

    ct_full = np.ascontiguousarray(centers.T)
    maps = []
    ns_raw = N // n_cores
    ks = K // n_cores
    for r in range(n_cores):
        xpad = np.zeros((NS, D), dtype=np.float32)
        xpad[:ns_raw] = X[r * ns_raw : (r + 1) * ns_raw]
        wpad = np.zeros((NS,), dtype=np.float32)
        wpad[:ns_raw] = sample_weight[r * ns_raw : (r + 1) * ns_raw]
        maps.append(
            {
                "xt": np.ascontiguousarray(xpad.T),
                "xn": xpad,
                "wv": wpad,
                "ct": ct_full,
                "cn": centers,
                "cm": np.ascontiguousarray(centers[r * ks : (r + 1) * ks]),
                "km": np.ascontiguousarray(counts[r * ks : (r + 1) * ks]),
            }
        )
    return maps


def run(X, centers, counts, sample_weight, trace=False, **kw):
    from concourse.bass_utils import run_bass_kernel_spmd

    nc = _get_nc(NCORES)
    maps = _in_maps(X, centers, counts, sample_weight)
    res = run_bass_kernel_spmd(
        nc, maps, core_ids=list(range(NCORES)), trace=trace, **kw
    )
    ns_raw = N // NCORES
    centers_out = np.concatenate([r["centers_o"] for r in res.results], axis=0)
    counts_out = np.concatenate([r["counts_o"] for r in res.results], axis=0)
    assign = np.concatenate(
        [r["assign_o"][:ns_raw] for r in res.results], axis=0
    ).astype(np.int32)
    return (centers_out, counts_out, assign), res


def kernel(X, centers, counts, sample_weight):
    out, _ = run(X, centers, counts, sample_weight, trace=False)
    return out
